# revision 6
# baseline (speedup 1.0000x reference)
"""DCNet Trainium2 kernel — data-parallel over 8 NeuronCores.

Model (per reference):
    hidden = relu(relu(x @ W1 + b1) @ W2 + b2)                    # [B, 512]
    tb     = treat_basis(t)                                       # [B, 25]
    h1     = relu(einsum('bi,iod,bd->bo', hidden, dfc1_w, tb) + tb @ dfc1_b.T)
    Q      = einsum('bi,iod,bd->bo', h1, dfc2_w, tb) + tb @ dfc2_b.T
    returns (Q [B,1], hidden [B,512])

Strategy: shard batch B=16384 over 8 cores (2048 rows each), replicate weights.
Per core, activations are kept transposed ([feature, batch]) so the contraction
dim lands on SBUF partitions for the PE. The dominant op (dfc1, 215 GFLOP total)
runs as, per 128-row batch tile and per spline-basis index d:
    psum_d[b,o] += hiddenT[i_chunk, b].T @ dfc1_w[i_chunk, :, d]   (4 K-chunks)
    acc[b,o]    = psum_d * tb[b,d] + acc          (fused DVE scalar_tensor_tensor)
Q is then rowsum(relu(acc) * u) where u = tb @ dfc2_w[:,0,:].T computed by PE.

Matmul operands are bf16 (fp32 PSUM accumulation); tb scaling stays fp32.
"""
import numpy as np
import ml_dtypes

B = 16384
NCORES = 8
BC = B // NCORES          # 2048 rows per core
NBT = BC // 128           # 16 batch tiles per core
COV = 256
H = 512
DD = 25
KNOTS = [0.33, 0.66]
DEGREE = 2

_BF16 = ml_dtypes.bfloat16

_CACHE = {}


def _treat_basis(t):
    """Truncated power basis + kron, matching the reference (fp32 numpy)."""
    t = t.astype(np.float32)
    knots = np.asarray([KNOTS, KNOTS], dtype=np.float32)              # [2, K]
    powers = np.stack([t ** p for p in range(DEGREE + 1)], axis=-1)   # [B, 2, 3]
    rel = np.maximum(t[..., None] - knots[None], 0.0) ** DEGREE       # [B, 2, 2]
    basis = np.concatenate([powers, rel], axis=-1)                    # [B, 2, 5]
    tb = np.einsum('bi,bj->bij', basis[:, 0], basis[:, 1])
    return tb.reshape(t.shape[0], -1)                                 # [B, 25]


def _build_nc():
    if "nc" in _CACHE:
        return _CACHE["nc"]
    from concourse import bacc, mybir
    import concourse.tile as tile

    BF16 = mybir.dt.bfloat16
    F32 = mybir.dt.float32
    AF = mybir.ActivationFunctionType
    OP = mybir.AluOpType

    nc = bacc.Bacc("TRN2", target_bir_lowering=False, debug=False,
                   num_devices=NCORES)

    xt_d = nc.dram_tensor("xt", [2, 128, BC], BF16, kind="ExternalInput").ap()
    w1_d = nc.dram_tensor("w1", [2, 128, H], BF16, kind="ExternalInput").ap()
    w2_d = nc.dram_tensor("w2", [4, 128, H], BF16, kind="ExternalInput").ap()
    b1_d = nc.dram_tensor("b1c", [128, 4], F32, kind="ExternalInput").ap()
    b2_d = nc.dram_tensor("b2c", [128, 4], F32, kind="ExternalInput").ap()
    wt_d = nc.dram_tensor("wt", [4, 128, DD, H], BF16, kind="ExternalInput").ap()
    d1b_d = nc.dram_tensor("d1bT", [DD, H], BF16, kind="ExternalInput").ap()
    tbp_d = nc.dram_tensor("tbp", [128, NBT, DD], F32, kind="ExternalInput").ap()
    tbt_d = nc.dram_tensor("tbt", [DD, BC], BF16, kind="ExternalInput").ap()
    w2d_d = nc.dram_tensor("w2d", [DD, 516], BF16, kind="ExternalInput").ap()

    hid_d = nc.dram_tensor("hid", [4, 128, BC], F32, kind="ExternalOutput").ap()
    q_d = nc.dram_tensor("q", [128, NBT], F32, kind="ExternalOutput").ap()

    # d-group schedule for the dfc1 accumulation: 26 psum slots
    # (1 bias + 25 basis indices) in groups of <=4 (one [128, 2048] psum tile
    # = 4 banks each, double-buffered = all 8 banks).
    slots = [None] + list(range(DD))     # None = dfc1 bias matmul
    groups = [slots[i:i + 4] for i in range(0, 26, 4)]   # 6*4 + 2

    with tile.TileContext(nc) as tc:
        with (
            tc.tile_pool(name="const", bufs=1) as cp,
            tc.tile_pool(name="work", bufs=2) as wp,
        ):
            # small tensors first so phase 1 / phase 2 heads are not stuck
            # behind the 13 MB dfc1 weight load
            xt = cp.tile([128, 2, BC], BF16)
            for ic in range(2):
                nc.sync.dma_start(xt[:, ic, :], xt_d[ic])
            w1 = cp.tile([128, 2, H], BF16)
            for ic in range(2):
                nc.sync.dma_start(w1[:, ic, :], w1_d[ic])
            w2 = cp.tile([128, 4, H], BF16)
            for ic in range(4):
                nc.sync.dma_start(w2[:, ic, :], w2_d[ic])
            b1s = cp.tile([128, 4], F32)
            nc.sync.dma_start(b1s[:], b1_d)
            b2s = cp.tile([128, 4], F32)
            nc.sync.dma_start(b2s[:], b2_d)
            d1b = cp.tile([DD, H], BF16)
            nc.sync.dma_start(d1b[:], d1b_d)
            tbp = cp.tile([128, NBT, DD], F32)
            nc.sync.dma_start(tbp[:], tbp_d)
            tbt = cp.tile([DD, BC], BF16)
            nc.sync.dma_start(tbt[:], tbt_d)
            w2d = cp.tile([DD, 516], BF16)
            nc.sync.dma_start(w2d[:], w2d_d)
            # dfc1 weights, split by d-block (block-major order) so the first
            # d-groups of phase 2 can start before the whole 13 MB arrives
            DBLK = [(0, 7), (7, 13), (13, 19), (19, 25)]
            wt_t = {}
            for bi, (d0, d1_) in enumerate(DBLK):
                for ic in range(4):
                    w_ic = cp.tile([128, d1_ - d0, H], BF16, tag=f"wt{ic}_{bi}")
                    nc.sync.dma_start(w_ic[:], wt_d[ic][:, d0:d1_, :])
                    wt_t[(ic, bi)] = w_ic

            def wt_rhs(ic, d):
                for bi, (d0, d1_) in enumerate(DBLK):
                    if d0 <= d < d1_:
                        return wt_t[(ic, bi)][:, d - d0, :]
                raise AssertionError

            h1t = cp.tile([128, 4, BC], BF16)
            ht = cp.tile([128, 4, BC], BF16)
            qout = cp.tile([128, NBT], F32)

            # single psum pool shared by both phases (no pool boundary, so the
            # scheduler can overlap the phase-1 tail with the phase-2 head)
            with tc.tile_pool(name="psum", bufs=2, space="PSUM") as pp:
                # ---- phase 1: representation MLP (transposed activations) ----
                with nc.named_scope("mlp1"):
                    for bc_i in range(4):
                        bsl = slice(bc_i * 512, (bc_i + 1) * 512)
                        for ot in range(4):
                            osl = slice(ot * 128, (ot + 1) * 128)
                            ps = pp.tile([128, 2048], F32, tag="ps")
                            for ic in range(2):
                                nc.tensor.matmul(ps[:, 0:512], w1[:, ic, osl],
                                                 xt[:, ic, bsl],
                                                 start=(ic == 0), stop=(ic == 1))
                            nc.scalar.activation(h1t[:, ot, bsl], ps[:, 0:512],
                                                 AF.Relu, bias=b1s[:, ot:ot + 1])
                with nc.named_scope("mlp2"):
                    for bc_i in range(4):
                        bsl = slice(bc_i * 512, (bc_i + 1) * 512)
                        for ot in range(4):
                            osl = slice(ot * 128, (ot + 1) * 128)
                            ps = pp.tile([128, 2048], F32, tag="ps")
                            for ic in range(4):
                                nc.tensor.matmul(ps[:, 0:512], w2[:, ic, osl],
                                                 h1t[:, ic, bsl],
                                                 start=(ic == 0), stop=(ic == 3))
                            nc.scalar.activation(ht[:, ot, bsl], ps[:, 0:512],
                                                 AF.Relu, bias=b2s[:, ot:ot + 1])
                            hs = wp.tile([128, 512], F32, tag="hstage")
                            nc.scalar.activation(hs[:], ps[:, 0:512], AF.Relu,
                                                 bias=b2s[:, ot:ot + 1])
                            nc.sync.dma_start(hid_d[ot][:, bsl], hs[:])

                # ---- phase 2: dynamic FC layers ----
                with nc.named_scope("dfc"):
                    for bt in range(NBT):
                        bsl = slice(bt * 128, (bt + 1) * 128)
                        acc = wp.tile([128, 512], F32, tag="acc")
                        for group in groups:
                            ps = pp.tile([128, 2048], F32, tag="ps")
                            for ic in range(4):
                                for j, d in enumerate(group):
                                    psl = ps[:, j * 512:(j + 1) * 512]
                                    if d is None:
                                        if ic == 0:
                                            nc.tensor.matmul(
                                                psl, tbt[:, bsl], d1b[:],
                                                start=True, stop=True)
                                    else:
                                        nc.tensor.matmul(
                                            psl, ht[:, ic, bsl], wt_rhs(ic, d),
                                            start=(ic == 0), stop=(ic == 3))
                            for j, d in enumerate(group):
                                psl = ps[:, j * 512:(j + 1) * 512]
                                if d is None:
                                    nc.vector.tensor_copy(acc[:], psl)
                                else:
                                    nc.vector.scalar_tensor_tensor(
                                        acc[:], psl, tbp[:, bt, d:d + 1], acc[:],
                                        OP.mult, OP.add)
                        # h1 = relu(acc); Q = rowsum(h1 * u) + u_bias
                        h1b = wp.tile([128, 512], BF16, tag="h1b")
                        nc.scalar.activation(h1b[:], acc[:], AF.Relu)
                        ups = pp.tile([128, 2048], F32, tag="ps")
                        nc.tensor.matmul(ups[:, 0:512], tbt[:, bsl],
                                         w2d[:, 0:512], start=True, stop=True)
                        nc.tensor.matmul(ups[:, 512:513], tbt[:, bsl],
                                         w2d[:, 512:513], start=True, stop=True)
                        vt = wp.tile([128, 512], F32, tag="vt")
                        qraw = wp.tile([128, 1], F32, tag="qraw")
                        nc.vector.scalar_tensor_tensor(
                            vt[:], ups[:, 0:512], 1.0, h1b[:],
                            OP.mult, OP.mult, accum_out=qraw[:])
                        nc.vector.tensor_add(qout[:, bt:bt + 1], qraw[:],
                                             ups[:, 512:513])
                    nc.sync.dma_start(q_d[:], qout[:])

    nc.finalize()
    _CACHE["nc"] = nc
    return nc


def _prepare(t, x, W1, b1, W2, b2, dfc1_w, dfc1_b, dfc2_w, dfc2_b):
    t = np.asarray(t, dtype=np.float32)
    x = np.asarray(x, dtype=np.float32)
    W1 = np.asarray(W1, dtype=np.float32)
    b1 = np.asarray(b1, dtype=np.float32)
    W2 = np.asarray(W2, dtype=np.float32)
    b2 = np.asarray(b2, dtype=np.float32)
    dfc1_w = np.asarray(dfc1_w, dtype=np.float32)
    dfc1_b = np.asarray(dfc1_b, dtype=np.float32)
    dfc2_w = np.asarray(dfc2_w, dtype=np.float32)
    dfc2_b = np.asarray(dfc2_b, dtype=np.float32)

    # replicated weights (host-side relayouts)
    w1 = np.ascontiguousarray(W1.reshape(2, 128, H)).astype(_BF16)
    w2 = np.ascontiguousarray(W2.reshape(4, 128, H)).astype(_BF16)
    b1c = np.ascontiguousarray(b1.reshape(4, 128).T)
    b2c = np.ascontiguousarray(b2.reshape(4, 128).T)
    wt = np.ascontiguousarray(
        dfc1_w.reshape(4, 128, H, DD).transpose(0, 1, 3, 2)).astype(_BF16)
    d1bT = np.ascontiguousarray(dfc1_b.T).astype(_BF16)          # [25, 512]
    w2d = np.zeros((DD, 516), np.float32)
    w2d[:, :512] = dfc2_w[:, 0, :].T
    w2d[:, 512] = dfc2_b[0]
    w2d = w2d.astype(_BF16)

    tb = _treat_basis(t)                                          # [B, 25] f32

    in_maps = []
    for c in range(NCORES):
        rs = slice(c * BC, (c + 1) * BC)
        xs = x[rs]
        tbs = tb[rs]
        in_maps.append(dict(
            xt=np.ascontiguousarray(xs.T.reshape(2, 128, BC)).astype(_BF16),
            w1=w1, w2=w2, b1c=b1c, b2c=b2c, wt=wt, d1bT=d1bT,
            tbp=np.ascontiguousarray(tbs.reshape(NBT, 128, DD).transpose(1, 0, 2)),
            tbt=np.ascontiguousarray(tbs.T).astype(_BF16),
            w2d=w2d,
        ))
    return in_maps


def _gather(res):
    Q = np.empty((B, 1), np.float32)
    hidden = np.empty((B, H), np.float32)
    for c in range(NCORES):
        rs = slice(c * BC, (c + 1) * BC)
        hid = res.results[c]["hid"]                # [4, 128, BC]
        hidden[rs] = hid.reshape(H, BC).T
        q = res.results[c]["q"]                    # [128, NBT]
        Q[rs, 0] = q.T.reshape(BC)
    return (Q, hidden)


def kernel(t, x, W1, b1, W2, b2, dfc1_w, dfc1_b, dfc2_w, dfc2_b):
    from concourse.bass_utils import run_bass_kernel_spmd

    in_maps = _prepare(t, x, W1, b1, W2, b2, dfc1_w, dfc1_b, dfc2_w, dfc2_b)
    nc = _build_nc()
    res = run_bass_kernel_spmd(nc, in_maps, core_ids=list(range(NCORES)))
    return _gather(res)


# revision 32
# speedup vs baseline: 1.5371x; 1.5371x over previous
"""DCNet Trainium2 kernel — data-parallel over 8 NeuronCores.

Model (per reference):
    hidden = relu(relu(x @ W1 + b1) @ W2 + b2)                    # [B, 512]
    tb     = treat_basis(t)                                       # [B, 25]
    h1     = relu(einsum('bi,iod,bd->bo', hidden, dfc1_w, tb) + tb @ dfc1_b.T)
    Q      = einsum('bi,iod,bd->bo', h1, dfc2_w, tb) + tb @ dfc2_b.T
    returns (Q [B,1], hidden [B,512])

Strategy: shard batch B=16384 over 8 cores (2048 rows each), replicate weights.
Per core, activations are kept transposed ([feature, batch]) so the contraction
dim lands on SBUF partitions for the PE.

The dominant op (dfc1, 215 GFLOP total) runs with fp8e4m3 DoubleRow matmuls
(hidden scaled x256 into fp8, weights fp8, fp32 PSUM accumulation; the
non-negative coherent sums wash the fp8 rounding out to ~2e-3):
    psum_d[b,o] += hiddenT[k-pair, b].T @ dfc1_w[k-pair, :, d]   (2 DoubleRow MMs)
and the spline-basis weighting drains PSUM on two engines in parallel,
d-group-major so compute overlaps the weight DMA stream:
    ScalarE:  tmp_d = psum_d * (tb[b,d]/256)        (scaled copy -> fp16)
    VectorE:  acc  += tmp_d                          (fp16 2x-mode adds)
              acc   = psum_d * tb_d/256 + acc        (fused scalar_tensor_tensor)
The dfc1 bias rides in slot d=0 (tb[:,0] == 1) as a 256x-scaled matmul.
Q is then rowsum(relu(acc) * u) + tb@dfc2_b, with u = tb @ dfc2_w[:,0,:].T on
the PE and the row-sum from scalar_tensor_tensor's accum_out. The MLP runs in
bf16; `hidden` is returned at bf16 precision.
"""
import numpy as np
import ml_dtypes

B = 16384
NCORES = 8
BC = B // NCORES          # 2048 rows per core
NBT = BC // 128           # 16 batch tiles per core
COV = 256
H = 512
DD = 25
KNOTS = [0.33, 0.66]
DEGREE = 2

_BF16 = ml_dtypes.bfloat16
_FP8 = ml_dtypes.float8_e4m3

_CACHE = {}


def _treat_basis(t):
    """Truncated power basis + kron, matching the reference (fp32 numpy)."""
    t = t.astype(np.float32)
    knots = np.asarray([KNOTS, KNOTS], dtype=np.float32)              # [2, K]
    powers = np.stack([t ** p for p in range(DEGREE + 1)], axis=-1)   # [B, 2, 3]
    rel = np.maximum(t[..., None] - knots[None], 0.0) ** DEGREE       # [B, 2, 2]
    basis = np.concatenate([powers, rel], axis=-1)                    # [B, 2, 5]
    tb = np.einsum('bi,bj->bij', basis[:, 0], basis[:, 1])
    return tb.reshape(t.shape[0], -1)                                 # [B, 25]


def _build_nc():
    if "nc" in _CACHE:
        return _CACHE["nc"]
    from concourse import bacc, mybir
    import concourse.tile as tile

    BF16 = mybir.dt.bfloat16
    FP8 = mybir.dt.float8e4
    F32 = mybir.dt.float32
    AF = mybir.ActivationFunctionType
    OP = mybir.AluOpType
    DR = mybir.MatmulPerfMode.DoubleRow
    F16 = mybir.dt.float16

    nc = bacc.Bacc("TRN2", target_bir_lowering=False, debug=False,
                   num_devices=NCORES)

    xt_d = nc.dram_tensor("xt", [2, 128, BC], BF16, kind="ExternalInput").ap()
    w1_d = nc.dram_tensor("w1", [2, 128, H], BF16, kind="ExternalInput").ap()
    w2_d = nc.dram_tensor("w2", [4, 128, H], BF16, kind="ExternalInput").ap()
    b1_d = nc.dram_tensor("b1c", [128, 4], F32, kind="ExternalInput").ap()
    # b2c columns 0-3: b2; columns 4-7: 256*b2 (bias for the fp8-scaled copy)
    b2_d = nc.dram_tensor("b2c", [128, 8], F32, kind="ExternalInput").ap()
    # dfc1_w in DoubleRow-interleaved fp8: [icp, p, j, d, o], k = icp*256+j*128+p
    wt_d = nc.dram_tensor("wt", [2, 128, 2, DD, H], FP8, kind="ExternalInput").ap()
    d1b_d = nc.dram_tensor("d1bT", [DD, H], BF16, kind="ExternalInput").ap()
    tbp_d = nc.dram_tensor("tbp", [128, NBT, DD + 1], F32, kind="ExternalInput").ap()
    tbt_d = nc.dram_tensor("tbt", [DD, BC], BF16, kind="ExternalInput").ap()
    w2d_d = nc.dram_tensor("w2d", [DD, 516], BF16, kind="ExternalInput").ap()

    hid_d = nc.dram_tensor("hid", [4, 128, BC], BF16, kind="ExternalOutput").ap()
    q_d = nc.dram_tensor("q", [128, NBT], F32, kind="ExternalOutput").ap()

    # d-group schedule for the dfc1 accumulation: 26 psum slots
    # (1 bias + 25 basis indices) in groups of 2 (one [128, 1024] psum tile
    # = 2 banks, triple-buffered = 6 banks; 2 banks left for the u matmuls).
    # The drain chain is 3 stages deep (PE fill -> ACT scaled copy -> DVE add),
    # so triple buffering is needed to hide the chain latency.
    # tb[:, 0] == 1 identically, so the dfc1 bias matmul (x256 on host) is
    # accumulated straight into slot d=0 and picked up by its 1/256 drain.
    slots = list(range(DD))              # 25 slots, bias folded into slot 0
    groups = [slots[i:i + 2] for i in range(0, DD, 2)]   # 12*2 + 1
    # which groups keep a fused DVE scalar_tensor_tensor drain (6 of 13);
    # the rest drain via ScalarE scaled-copy + DVE fp16 add (19 ACT slots)
    dve_group = [1, 0, 1, 0, 1, 0, 1, 0, 1, 0, 1, 0, 1]

    with tile.TileContext(nc) as tc:
        with (
            tc.tile_pool(name="const", bufs=1) as cp,
            tc.tile_pool(name="work", bufs=2) as wp,
        ):
            # small tensors first so phase 1 / phase 2 heads are not stuck
            # behind the 13 MB dfc1 weight load
            w1 = cp.tile([128, 2, H], BF16)
            for ic in range(2):
                nc.sync.dma_start(w1[:, ic, :], w1_d[ic])
            xt = cp.tile([128, 2, BC], BF16)
            for bc_i in range(4):
                for ic in range(2):
                    nc.sync.dma_start(xt[:, ic, bc_i * 512:(bc_i + 1) * 512],
                                      xt_d[ic][:, bc_i * 512:(bc_i + 1) * 512])
            w2 = cp.tile([128, 4, H], BF16)
            for ic in range(4):
                nc.sync.dma_start(w2[:, ic, :], w2_d[ic])
            b1s = cp.tile([128, 4], F32)
            nc.sync.dma_start(b1s[:], b1_d)
            b2s = cp.tile([128, 8], F32)
            nc.sync.dma_start(b2s[:], b2_d)
            d1b = cp.tile([DD, H], BF16)
            nc.sync.dma_start(d1b[:], d1b_d)
            tbp = cp.tile([128, NBT, DD + 1], F32)
            nc.sync.dma_start(tbp[:], tbp_d)
            tbt = cp.tile([DD, BC], BF16)
            nc.sync.dma_start(tbt[:], tbt_d)
            w2d = cp.tile([DD, 516], BF16)
            nc.sync.dma_start(w2d[:], w2d_d)
            # dfc1 weights, split by d-block (block-major order) so the first
            # d-groups of phase 2 can start before the whole 6.5 MB arrives
            DBLK = [(0, 7), (7, 13), (13, 19), (19, 25)]
            wt_t = {}
            for bi, (d0, d1_) in enumerate(DBLK):
                for icp in range(2):
                    w_ic = cp.tile([128, 2, d1_ - d0, H], FP8, tag=f"wt{icp}_{bi}")
                    nc.sync.dma_start(w_ic[:], wt_d[icp][:, :, d0:d1_, :])
                    wt_t[(icp, bi)] = w_ic

            def wt_rhs(icp, d):
                for bi, (d0, d1_) in enumerate(DBLK):
                    if d0 <= d < d1_:
                        return wt_t[(icp, bi)][:, :, d - d0, :]
                raise AssertionError

            ht = cp.tile([128, 4, BC], BF16)
            ht8 = cp.tile([128, 4, BC], FP8)   # 256*hidden, DoubleRow lhsT
            qout = cp.tile([128, NBT], F32)

            # single psum pool shared by both phases (no pool boundary, so the
            # scheduler can overlap the phase-1 tail with the phase-2 head)
            with (
                tc.tile_pool(name="psum", bufs=3, space="PSUM") as pp,
                tc.tile_pool(name="psumu", bufs=2, space="PSUM") as ppu,
            ):
                # ---- phase 1: representation MLP (transposed activations) ----
                # h1t lives in its own pool that closes after layer 2, so the
                # phase-2 accumulators reuse its SBUF space.
                with tc.tile_pool(name="ph1", bufs=1) as ph1:
                    h1t = ph1.tile([128, 4, BC], BF16)
                    with nc.named_scope("mlp1"):
                        for bc_i in range(4):
                            bsl = slice(bc_i * 512, (bc_i + 1) * 512)
                            for ot in range(4):
                                osl = slice(ot * 128, (ot + 1) * 128)
                                ps = pp.tile([128, 1024], F32, tag="ps")
                                for ic in range(2):
                                    nc.tensor.matmul(ps[:, 0:512], w1[:, ic, osl],
                                                     xt[:, ic, bsl],
                                                     start=(ic == 0), stop=(ic == 1))
                                nc.scalar.activation(h1t[:, ot, bsl], ps[:, 0:512],
                                                     AF.Relu, bias=b1s[:, ot:ot + 1])
                    with nc.named_scope("mlp2"):
                        for bc_i in range(4):
                            bsl = slice(bc_i * 512, (bc_i + 1) * 512)
                            for ot in range(4):
                                osl = slice(ot * 128, (ot + 1) * 128)
                                ps = pp.tile([128, 1024], F32, tag="ps")
                                for ic in range(4):
                                    nc.tensor.matmul(ps[:, 0:512], w2[:, ic, osl],
                                                     h1t[:, ic, bsl],
                                                     start=(ic == 0), stop=(ic == 3))
                                nc.scalar.activation(ht[:, ot, bsl], ps[:, 0:512],
                                                     AF.Relu, bias=b2s[:, ot:ot + 1])
                                nc.scalar.activation(ht8[:, ot, bsl], ps[:, 0:512],
                                                     AF.Relu, scale=256.0,
                                                     bias=b2s[:, 4 + ot:5 + ot])
                                nc.sync.dma_start(hid_d[ot][:, bsl],
                                                  ht[:, ot, bsl])

                # ---- phase 2: dynamic FC layers, d-group-major ----
                # All 16 batch tiles advance one d-group at a time, so group g
                # only needs d-block g's weights: compute overlaps the 13 MB
                # dfc1 weight stream instead of stalling on it.
                with tc.tile_pool(name="paccs", bufs=1) as paccs:
                    accs = paccs.tile([128, NBT, 512], F16)
                    with nc.named_scope("dfc"):
                        for gi, group in enumerate(groups):
                            # drain split: one d per (full) group stays a fused
                            # DVE scalar_tensor_tensor; the others go to
                            # ScalarE (scaled psum->fp16 copy) + cheap fp16
                            # 2x-mode DVE adds. 8 DVE-fused + 18 ACT slots
                            # balances ACT/DVE against the PE fill rate.
                            dve_d = group[-1] if dve_group[gi] else None
                            for bt in range(NBT):
                                bsl = slice(bt * 128, (bt + 1) * 128)
                                acc = accs[:, bt, :]
                                ps = pp.tile([128, 1024], F32, tag="ps")
                                for icp in range(2):
                                    for j, d in enumerate(group):
                                        psl = ps[:, j * 512:(j + 1) * 512]
                                        if d == 0 and icp == 0:
                                            # 256*dfc1_b, same 1/256 drain scale
                                            nc.tensor.matmul(
                                                psl, tbt[:, bsl], d1b[:],
                                                start=True, stop=False)
                                        nc.tensor.matmul(
                                            psl,
                                            ht8[:, 2 * icp:2 * icp + 2, bsl],
                                            wt_rhs(icp, d),
                                            start=(icp == 0 and d != 0),
                                            stop=(icp == 1),
                                            perf_mode=DR)
                                tmps = []
                                for j, d in enumerate(group):
                                    psl = ps[:, j * 512:(j + 1) * 512]
                                    if d != dve_d:
                                        tmp = wp.tile([128, 512], F16,
                                                      tag=f"tmp{j}")
                                        nc.scalar.activation(
                                            tmp[:], psl, AF.Copy,
                                            scale=tbp[:, bt, d:d + 1])
                                        tmps.append((j, tmp))
                                if gi == 0 and bt == 0:
                                    pass
                                for j, d in enumerate(group):
                                    if d == dve_d:
                                        psl = ps[:, j * 512:(j + 1) * 512]
                                        if gi == 0:
                                            # first touch of acc this bt
                                            nc.vector.tensor_scalar(
                                                acc, psl, tbp[:, bt, d:d + 1],
                                                None, OP.mult)
                                        else:
                                            nc.vector.scalar_tensor_tensor(
                                                acc, psl, tbp[:, bt, d:d + 1],
                                                acc, OP.mult, OP.add)
                                first = (gi == 0 and not dve_group[0])
                                for j, tmp in tmps:
                                    if first:
                                        nc.vector.tensor_copy(acc, tmp[:])
                                        first = False
                                    else:
                                        nc.vector.tensor_add(acc, acc, tmp[:])
                                if gi == len(groups) - 1:
                                    # Q epilogue, fused so it overlaps the
                                    # remaining batch tiles' last groups
                                    h1b = wp.tile([128, 512], F16, tag="h1b")
                                    nc.scalar.activation(h1b[:], acc, AF.Relu)
                                    ups = ppu.tile([128, 512], F32, tag="ups")
                                    nc.tensor.matmul(ups[:], tbt[:, bsl],
                                                     w2d[:, 0:512],
                                                     start=True, stop=True)
                                    vt = wp.tile([128, 512], F32, tag="vt")
                                    qraw = wp.tile([128, 1], F32, tag="qraw")
                                    nc.vector.scalar_tensor_tensor(
                                        vt[:], ups[:], 1.0, h1b[:],
                                        OP.mult, OP.mult, accum_out=qraw[:])
                                    nc.vector.tensor_add(qout[:, bt:bt + 1],
                                                         qraw[:],
                                                         tbp[:, bt, DD:DD + 1])
                        nc.sync.dma_start(q_d[:], qout[:])

    nc.finalize()
    _CACHE["nc"] = nc
    return nc


def _prepare(t, x, W1, b1, W2, b2, dfc1_w, dfc1_b, dfc2_w, dfc2_b):
    t = np.asarray(t, dtype=np.float32)
    x = np.asarray(x, dtype=np.float32)
    W1 = np.asarray(W1, dtype=np.float32)
    b1 = np.asarray(b1, dtype=np.float32)
    W2 = np.asarray(W2, dtype=np.float32)
    b2 = np.asarray(b2, dtype=np.float32)
    dfc1_w = np.asarray(dfc1_w, dtype=np.float32)
    dfc1_b = np.asarray(dfc1_b, dtype=np.float32)
    dfc2_w = np.asarray(dfc2_w, dtype=np.float32)
    dfc2_b = np.asarray(dfc2_b, dtype=np.float32)

    # replicated weights (host-side relayouts)
    w1 = np.ascontiguousarray(W1.reshape(2, 128, H)).astype(_BF16)
    w2 = np.ascontiguousarray(W2.reshape(4, 128, H)).astype(_BF16)
    b1c = np.ascontiguousarray(b1.reshape(4, 128).T)
    b2q = b2.reshape(4, 128).T
    b2c = np.ascontiguousarray(np.concatenate([b2q, 256.0 * b2q], axis=1))
    # DoubleRow-interleaved fp8 dfc1_w: [icp, p, j, d, o], k = icp*256+j*128+p
    wt = np.ascontiguousarray(
        dfc1_w.reshape(2, 2, 128, H, DD).transpose(0, 2, 1, 4, 3)
    ).astype(_FP8)
    d1bT = np.ascontiguousarray(256.0 * dfc1_b.T).astype(_BF16)  # [25, 512]
    w2d = np.zeros((DD, 516), np.float32)
    w2d[:, :512] = dfc2_w[:, 0, :].T
    w2d[:, 512] = dfc2_b[0]
    w2d = w2d.astype(_BF16)

    tb = _treat_basis(t)                                          # [B, 25] f32

    in_maps = []
    for c in range(NCORES):
        rs = slice(c * BC, (c + 1) * BC)
        xs = x[rs]
        tbs = tb[rs]
        in_maps.append(dict(
            xt=np.ascontiguousarray(xs.T.reshape(2, 128, BC)).astype(_BF16),
            w1=w1, w2=w2, b1c=b1c, b2c=b2c, wt=wt, d1bT=d1bT,
            tbp=np.concatenate([
                np.ascontiguousarray(
                    tbs.reshape(NBT, 128, DD).transpose(1, 0, 2)) / 256.0,
                (tbs @ dfc2_b[0]).reshape(NBT, 128, 1).transpose(1, 0, 2),
            ], axis=2),
            tbt=np.ascontiguousarray(tbs.T).astype(_BF16),
            w2d=w2d,
        ))
    return in_maps


def _gather(res):
    Q = np.empty((B, 1), np.float32)
    hidden = np.empty((B, H), np.float32)
    for c in range(NCORES):
        rs = slice(c * BC, (c + 1) * BC)
        hid = res.results[c]["hid"]                # [4, 128, BC] bf16
        hidden[rs] = hid.reshape(H, BC).T.astype(np.float32)
        q = res.results[c]["q"]                    # [128, NBT]
        Q[rs, 0] = q.T.reshape(BC)
    return (Q, hidden)


def kernel(t, x, W1, b1, W2, b2, dfc1_w, dfc1_b, dfc2_w, dfc2_b):
    from concourse.bass_utils import run_bass_kernel_spmd

    in_maps = _prepare(t, x, W1, b1, W2, b2, dfc1_w, dfc1_b, dfc2_w, dfc2_b)
    nc = _build_nc()
    res = run_bass_kernel_spmd(nc, in_maps, core_ids=list(range(NCORES)))
    return _gather(res)


# revision 33
# speedup vs baseline: 1.6613x; 1.0808x over previous
"""DCNet Trainium2 kernel — data-parallel over 8 NeuronCores.

Model (per reference):
    hidden = relu(relu(x @ W1 + b1) @ W2 + b2)                    # [B, 512]
    tb     = treat_basis(t)                                       # [B, 25]
    h1     = relu(einsum('bi,iod,bd->bo', hidden, dfc1_w, tb) + tb @ dfc1_b.T)
    Q      = einsum('bi,iod,bd->bo', h1, dfc2_w, tb) + tb @ dfc2_b.T
    returns (Q [B,1], hidden [B,512])

Strategy: shard batch B=16384 over 8 cores (2048 rows each), replicate weights.
Per core, activations are kept transposed ([feature, batch]) so the contraction
dim lands on SBUF partitions for the PE.

The dominant op (dfc1, 215 GFLOP total) runs with fp8e4m3 DoubleRow matmuls
(hidden scaled x256 into fp8, weights fp8, fp32 PSUM accumulation; the
non-negative coherent sums wash the fp8 rounding out to ~2e-3):
    psum_d[b,o] += hiddenT[k-pair, b].T @ dfc1_w[k-pair, :, d]   (2 DoubleRow MMs)
and the spline-basis weighting drains PSUM on two engines in parallel,
d-group-major so compute overlaps the weight DMA stream:
    ScalarE:  tmp_d = psum_d * (tb[b,d]/256)        (scaled copy -> fp16)
    VectorE:  acc  += tmp_d                          (fp16 2x-mode adds)
              acc   = psum_d * tb_d/256 + acc        (fused scalar_tensor_tensor)
The dfc1 bias rides in slot d=0 (tb[:,0] == 1) as a 256x-scaled matmul.
Q is then rowsum(relu(acc) * u) + tb@dfc2_b, with u = tb @ dfc2_w[:,0,:].T on
the PE and the row-sum from scalar_tensor_tensor's accum_out. The MLP runs in
bf16; `hidden` is returned at bf16 precision.
"""
import numpy as np
import ml_dtypes

B = 16384
NCORES = 8
BC = B // NCORES          # 2048 rows per core
NBT = BC // 128           # 16 batch tiles per core
COV = 256
H = 512
DD = 25
KNOTS = [0.33, 0.66]
DEGREE = 2

_BF16 = ml_dtypes.bfloat16
_FP8 = ml_dtypes.float8_e4m3

_CACHE = {}


def _treat_basis(t):
    """Truncated power basis + kron, matching the reference (fp32 numpy)."""
    t = t.astype(np.float32)
    knots = np.asarray([KNOTS, KNOTS], dtype=np.float32)              # [2, K]
    powers = np.stack([t ** p for p in range(DEGREE + 1)], axis=-1)   # [B, 2, 3]
    rel = np.maximum(t[..., None] - knots[None], 0.0) ** DEGREE       # [B, 2, 2]
    basis = np.concatenate([powers, rel], axis=-1)                    # [B, 2, 5]
    tb = np.einsum('bi,bj->bij', basis[:, 0], basis[:, 1])
    return tb.reshape(t.shape[0], -1)                                 # [B, 25]


def _build_nc():
    if "nc" in _CACHE:
        return _CACHE["nc"]
    from concourse import bacc, mybir
    import concourse.tile as tile

    BF16 = mybir.dt.bfloat16
    FP8 = mybir.dt.float8e4
    F32 = mybir.dt.float32
    AF = mybir.ActivationFunctionType
    OP = mybir.AluOpType
    DR = mybir.MatmulPerfMode.DoubleRow
    F16 = mybir.dt.float16

    nc = bacc.Bacc("TRN2", target_bir_lowering=False, debug=False,
                   num_devices=NCORES)

    xt_d = nc.dram_tensor("xt", [2, 128, BC], BF16, kind="ExternalInput").ap()
    w1_d = nc.dram_tensor("w1", [2, 128, H], BF16, kind="ExternalInput").ap()
    w2_d = nc.dram_tensor("w2", [4, 128, H], BF16, kind="ExternalInput").ap()
    b1_d = nc.dram_tensor("b1c", [128, 4], F32, kind="ExternalInput").ap()
    # b2c columns 0-3: b2; columns 4-7: 256*b2 (bias for the fp8-scaled copy)
    b2_d = nc.dram_tensor("b2c", [128, 8], F32, kind="ExternalInput").ap()
    # dfc1_w in DoubleRow-interleaved fp8: [icp, p, j, d, o], k = icp*256+j*128+p
    wt_d = nc.dram_tensor("wt", [2, 128, 2, DD, H], FP8, kind="ExternalInput").ap()
    d1b_d = nc.dram_tensor("d1bT", [DD, H], BF16, kind="ExternalInput").ap()
    tbp_d = nc.dram_tensor("tbp", [128, NBT, DD + 1], F32, kind="ExternalInput").ap()
    tbt_d = nc.dram_tensor("tbt", [DD, BC], BF16, kind="ExternalInput").ap()
    w2d_d = nc.dram_tensor("w2d", [DD, 516], BF16, kind="ExternalInput").ap()

    hid_d = nc.dram_tensor("hid", [4, 128, BC], BF16, kind="ExternalOutput").ap()
    q_d = nc.dram_tensor("q", [128, NBT], F32, kind="ExternalOutput").ap()

    # d-group schedule for the dfc1 accumulation: 26 psum slots
    # (1 bias + 25 basis indices) in groups of 2 (one [128, 1024] psum tile
    # = 2 banks, triple-buffered = 6 banks; 2 banks left for the u matmuls).
    # The drain chain is 3 stages deep (PE fill -> ACT scaled copy -> DVE add),
    # so triple buffering is needed to hide the chain latency.
    # tb[:, 0] == 1 identically, so the dfc1 bias matmul (x256 on host) is
    # accumulated straight into slot d=0 and picked up by its 1/256 drain.
    slots = list(range(DD))              # 25 slots, bias folded into slot 0
    groups = [slots[i:i + 2] for i in range(0, DD, 2)]   # 12*2 + 1
    # which groups keep a fused DVE scalar_tensor_tensor drain (6 of 13);
    # the rest drain via ScalarE scaled-copy + DVE fp16 add (19 ACT slots)
    dve_group = [1, 0, 1, 0, 1, 0, 1, 0, 1, 0, 1, 0, 1]

    with tile.TileContext(nc) as tc:
        with (
            tc.tile_pool(name="const", bufs=1) as cp,
            tc.tile_pool(name="work", bufs=2) as wp,
        ):
            # small tensors first so phase 1 / phase 2 heads are not stuck
            # behind the 13 MB dfc1 weight load
            w1 = cp.tile([128, 2, H], BF16)
            for ic in range(2):
                nc.sync.dma_start(w1[:, ic, :], w1_d[ic])
            xt = cp.tile([128, 2, BC], BF16)
            for bc_i in range(4):
                for ic in range(2):
                    nc.sync.dma_start(xt[:, ic, bc_i * 512:(bc_i + 1) * 512],
                                      xt_d[ic][:, bc_i * 512:(bc_i + 1) * 512])
            w2 = cp.tile([128, 4, H], BF16)
            for ic in range(4):
                nc.sync.dma_start(w2[:, ic, :], w2_d[ic])
            b1s = cp.tile([128, 4], F32)
            nc.sync.dma_start(b1s[:], b1_d)
            b2s = cp.tile([128, 8], F32)
            nc.sync.dma_start(b2s[:], b2_d)
            d1b = cp.tile([DD, H], BF16)
            nc.sync.dma_start(d1b[:], d1b_d)
            tbp = cp.tile([128, NBT, DD + 1], F32)
            nc.sync.dma_start(tbp[:], tbp_d)
            tbt = cp.tile([DD, BC], BF16)
            nc.sync.dma_start(tbt[:], tbt_d)
            w2d = cp.tile([DD, 516], BF16)
            nc.sync.dma_start(w2d[:], w2d_d)
            # dfc1 weights, split by d-block (block-major order) so the first
            # d-groups of phase 2 can start before the whole 6.5 MB arrives
            DBLK = [(0, 7), (7, 13), (13, 19), (19, 25)]
            wt_t = {}
            for bi, (d0, d1_) in enumerate(DBLK):
                for icp in range(2):
                    w_ic = cp.tile([128, 2, d1_ - d0, H], FP8, tag=f"wt{icp}_{bi}")
                    nc.sync.dma_start(w_ic[:], wt_d[icp][:, :, d0:d1_, :])
                    wt_t[(icp, bi)] = w_ic

            def wt_rhs(icp, d):
                for bi, (d0, d1_) in enumerate(DBLK):
                    if d0 <= d < d1_:
                        return wt_t[(icp, bi)][:, :, d - d0, :]
                raise AssertionError

            ht = cp.tile([128, 4, BC], BF16)
            ht8 = cp.tile([128, 4, BC], FP8)   # 256*hidden, DoubleRow lhsT
            qout = cp.tile([128, NBT], F32)

            # single psum pool shared by both phases (no pool boundary, so the
            # scheduler can overlap the phase-1 tail with the phase-2 head)
            with (
                tc.tile_pool(name="psum", bufs=3, space="PSUM") as pp,
                tc.tile_pool(name="psumu", bufs=2, space="PSUM") as ppu,
            ):
                # ---- phase 1: representation MLP (transposed activations) ----
                # h1t lives in its own pool that closes after layer 2, so the
                # phase-2 accumulators reuse its SBUF space.
                with tc.tile_pool(name="ph1", bufs=1) as ph1:
                    h1t = ph1.tile([128, 4, BC], BF16)
                    with nc.named_scope("mlp1"):
                        for bc_i in range(4):
                            bsl = slice(bc_i * 512, (bc_i + 1) * 512)
                            for ot in range(4):
                                osl = slice(ot * 128, (ot + 1) * 128)
                                ps = pp.tile([128, 1024], F32, tag="ps")
                                for ic in range(2):
                                    nc.tensor.matmul(ps[:, 0:512], w1[:, ic, osl],
                                                     xt[:, ic, bsl],
                                                     start=(ic == 0), stop=(ic == 1))
                                if ot % 2 == 0:
                                    nc.scalar.activation(
                                        h1t[:, ot, bsl], ps[:, 0:512],
                                        AF.Relu, bias=b1s[:, ot:ot + 1])
                                else:
                                    # relu(psum + b1) on DVE: (ps add b1) max 0
                                    nc.vector.tensor_scalar(
                                        h1t[:, ot, bsl], ps[:, 0:512],
                                        b1s[:, ot:ot + 1], 0.0,
                                        OP.add, OP.max)
                    with nc.named_scope("mlp2"):
                        for bc_i in range(4):
                            bsl = slice(bc_i * 512, (bc_i + 1) * 512)
                            for ot in range(4):
                                osl = slice(ot * 128, (ot + 1) * 128)
                                ps = pp.tile([128, 1024], F32, tag="ps")
                                for ic in range(4):
                                    nc.tensor.matmul(ps[:, 0:512], w2[:, ic, osl],
                                                     h1t[:, ic, bsl],
                                                     start=(ic == 0), stop=(ic == 3))
                                nc.vector.tensor_scalar(
                                    ht[:, ot, bsl], ps[:, 0:512],
                                    b2s[:, ot:ot + 1], 0.0, OP.add, OP.max)
                                nc.scalar.activation(ht8[:, ot, bsl], ps[:, 0:512],
                                                     AF.Relu, scale=256.0,
                                                     bias=b2s[:, 4 + ot:5 + ot])
                                nc.sync.dma_start(hid_d[ot][:, bsl],
                                                  ht[:, ot, bsl])

                # ---- phase 2: dynamic FC layers, d-group-major ----
                # All 16 batch tiles advance one d-group at a time, so group g
                # only needs d-block g's weights: compute overlaps the 13 MB
                # dfc1 weight stream instead of stalling on it.
                with tc.tile_pool(name="paccs", bufs=1) as paccs:
                    accs = paccs.tile([128, NBT, 512], F16)
                    with nc.named_scope("dfc"):
                        for gi, group in enumerate(groups):
                            # drain split: one d per (full) group stays a fused
                            # DVE scalar_tensor_tensor; the others go to
                            # ScalarE (scaled psum->fp16 copy) + cheap fp16
                            # 2x-mode DVE adds. 8 DVE-fused + 18 ACT slots
                            # balances ACT/DVE against the PE fill rate.
                            dve_d = group[-1] if dve_group[gi] else None
                            for bt in range(NBT):
                                bsl = slice(bt * 128, (bt + 1) * 128)
                                acc = accs[:, bt, :]
                                ps = pp.tile([128, 1024], F32, tag="ps")
                                for icp in range(2):
                                    for j, d in enumerate(group):
                                        psl = ps[:, j * 512:(j + 1) * 512]
                                        if d == 0 and icp == 0:
                                            # 256*dfc1_b, same 1/256 drain scale
                                            nc.tensor.matmul(
                                                psl, tbt[:, bsl], d1b[:],
                                                start=True, stop=False)
                                        nc.tensor.matmul(
                                            psl,
                                            ht8[:, 2 * icp:2 * icp + 2, bsl],
                                            wt_rhs(icp, d),
                                            start=(icp == 0 and d != 0),
                                            stop=(icp == 1),
                                            perf_mode=DR)
                                tmps = []
                                for j, d in enumerate(group):
                                    psl = ps[:, j * 512:(j + 1) * 512]
                                    if d != dve_d:
                                        tmp = wp.tile([128, 512], F16,
                                                      tag=f"tmp{j}")
                                        nc.scalar.activation(
                                            tmp[:], psl, AF.Copy,
                                            scale=tbp[:, bt, d:d + 1])
                                        tmps.append((j, tmp))
                                if gi == 0 and bt == 0:
                                    pass
                                for j, d in enumerate(group):
                                    if d == dve_d:
                                        psl = ps[:, j * 512:(j + 1) * 512]
                                        if gi == 0:
                                            # first touch of acc this bt
                                            nc.vector.tensor_scalar(
                                                acc, psl, tbp[:, bt, d:d + 1],
                                                None, OP.mult)
                                        else:
                                            nc.vector.scalar_tensor_tensor(
                                                acc, psl, tbp[:, bt, d:d + 1],
                                                acc, OP.mult, OP.add)
                                first = (gi == 0 and not dve_group[0])
                                for j, tmp in tmps:
                                    if first:
                                        nc.vector.tensor_copy(acc, tmp[:])
                                        first = False
                                    else:
                                        nc.vector.tensor_add(acc, acc, tmp[:])
                                if gi == len(groups) - 1:
                                    # Q epilogue, fused so it overlaps the
                                    # remaining batch tiles' last groups
                                    h1b = wp.tile([128, 512], F16, tag="h1b")
                                    nc.scalar.activation(h1b[:], acc, AF.Relu)
                                    ups = ppu.tile([128, 512], F32, tag="ups")
                                    nc.tensor.matmul(ups[:], tbt[:, bsl],
                                                     w2d[:, 0:512],
                                                     start=True, stop=True)
                                    vt = wp.tile([128, 512], F32, tag="vt")
                                    qraw = wp.tile([128, 1], F32, tag="qraw")
                                    nc.vector.scalar_tensor_tensor(
                                        vt[:], ups[:], 1.0, h1b[:],
                                        OP.mult, OP.mult, accum_out=qraw[:])
                                    nc.vector.tensor_add(qout[:, bt:bt + 1],
                                                         qraw[:],
                                                         tbp[:, bt, DD:DD + 1])
                        nc.sync.dma_start(q_d[:], qout[:])

    nc.finalize()
    _CACHE["nc"] = nc
    return nc


def _prepare(t, x, W1, b1, W2, b2, dfc1_w, dfc1_b, dfc2_w, dfc2_b):
    t = np.asarray(t, dtype=np.float32)
    x = np.asarray(x, dtype=np.float32)
    W1 = np.asarray(W1, dtype=np.float32)
    b1 = np.asarray(b1, dtype=np.float32)
    W2 = np.asarray(W2, dtype=np.float32)
    b2 = np.asarray(b2, dtype=np.float32)
    dfc1_w = np.asarray(dfc1_w, dtype=np.float32)
    dfc1_b = np.asarray(dfc1_b, dtype=np.float32)
    dfc2_w = np.asarray(dfc2_w, dtype=np.float32)
    dfc2_b = np.asarray(dfc2_b, dtype=np.float32)

    # replicated weights (host-side relayouts)
    w1 = np.ascontiguousarray(W1.reshape(2, 128, H)).astype(_BF16)
    w2 = np.ascontiguousarray(W2.reshape(4, 128, H)).astype(_BF16)
    b1c = np.ascontiguousarray(b1.reshape(4, 128).T)
    b2q = b2.reshape(4, 128).T
    b2c = np.ascontiguousarray(np.concatenate([b2q, 256.0 * b2q], axis=1))
    # DoubleRow-interleaved fp8 dfc1_w: [icp, p, j, d, o], k = icp*256+j*128+p
    wt = np.ascontiguousarray(
        dfc1_w.reshape(2, 2, 128, H, DD).transpose(0, 2, 1, 4, 3)
    ).astype(_FP8)
    d1bT = np.ascontiguousarray(256.0 * dfc1_b.T).astype(_BF16)  # [25, 512]
    w2d = np.zeros((DD, 516), np.float32)
    w2d[:, :512] = dfc2_w[:, 0, :].T
    w2d[:, 512] = dfc2_b[0]
    w2d = w2d.astype(_BF16)

    tb = _treat_basis(t)                                          # [B, 25] f32

    in_maps = []
    for c in range(NCORES):
        rs = slice(c * BC, (c + 1) * BC)
        xs = x[rs]
        tbs = tb[rs]
        in_maps.append(dict(
            xt=np.ascontiguousarray(xs.T.reshape(2, 128, BC)).astype(_BF16),
            w1=w1, w2=w2, b1c=b1c, b2c=b2c, wt=wt, d1bT=d1bT,
            tbp=np.concatenate([
                np.ascontiguousarray(
                    tbs.reshape(NBT, 128, DD).transpose(1, 0, 2)) / 256.0,
                (tbs @ dfc2_b[0]).reshape(NBT, 128, 1).transpose(1, 0, 2),
            ], axis=2),
            tbt=np.ascontiguousarray(tbs.T).astype(_BF16),
            w2d=w2d,
        ))
    return in_maps


def _gather(res):
    Q = np.empty((B, 1), np.float32)
    hidden = np.empty((B, H), np.float32)
    for c in range(NCORES):
        rs = slice(c * BC, (c + 1) * BC)
        hid = res.results[c]["hid"]                # [4, 128, BC] bf16
        hidden[rs] = hid.reshape(H, BC).T.astype(np.float32)
        q = res.results[c]["q"]                    # [128, NBT]
        Q[rs, 0] = q.T.reshape(BC)
    return (Q, hidden)


def kernel(t, x, W1, b1, W2, b2, dfc1_w, dfc1_b, dfc2_w, dfc2_b):
    from concourse.bass_utils import run_bass_kernel_spmd

    in_maps = _prepare(t, x, W1, b1, W2, b2, dfc1_w, dfc1_b, dfc2_w, dfc2_b)
    nc = _build_nc()
    res = run_bass_kernel_spmd(nc, in_maps, core_ids=list(range(NCORES)))
    return _gather(res)


# revision 34
# speedup vs baseline: 1.6620x; 1.0004x over previous
"""DCNet Trainium2 kernel — data-parallel over 8 NeuronCores.

Model (per reference):
    hidden = relu(relu(x @ W1 + b1) @ W2 + b2)                    # [B, 512]
    tb     = treat_basis(t)                                       # [B, 25]
    h1     = relu(einsum('bi,iod,bd->bo', hidden, dfc1_w, tb) + tb @ dfc1_b.T)
    Q      = einsum('bi,iod,bd->bo', h1, dfc2_w, tb) + tb @ dfc2_b.T
    returns (Q [B,1], hidden [B,512])

Strategy: shard batch B=16384 over 8 cores (2048 rows each), replicate weights.
Per core, activations are kept transposed ([feature, batch]) so the contraction
dim lands on SBUF partitions for the PE.

The dominant op (dfc1, 215 GFLOP total) runs with fp8e4m3 DoubleRow matmuls
(hidden scaled x256 into fp8, weights fp8, fp32 PSUM accumulation; the
non-negative coherent sums wash the fp8 rounding out to ~2e-3):
    psum_d[b,o] += hiddenT[k-pair, b].T @ dfc1_w[k-pair, :, d]   (2 DoubleRow MMs)
and the spline-basis weighting drains PSUM on two engines in parallel,
d-group-major so compute overlaps the weight DMA stream:
    ScalarE:  tmp_d = psum_d * (tb[b,d]/256)        (scaled copy -> fp16)
    VectorE:  acc  += tmp_d                          (fp16 2x-mode adds)
              acc   = psum_d * tb_d/256 + acc        (fused scalar_tensor_tensor)
The dfc1 bias rides in slot d=0 (tb[:,0] == 1) as a 256x-scaled matmul.
The MLP relu/bias drains are split across ScalarE and VectorE (DVE
tensor_scalar with fused add-bias + max-0), since DVE is otherwise idle in
phase 1 and ACT was head-of-line-blocking phase 2's copies.
Q is then rowsum(relu(acc) * u) + tb@dfc2_b, with u = tb @ dfc2_w[:,0,:].T on
the PE and the row-sum from scalar_tensor_tensor's accum_out. The MLP runs in
bf16; `hidden` is returned at bf16 precision.
"""
import numpy as np
import ml_dtypes

B = 16384
NCORES = 8
BC = B // NCORES          # 2048 rows per core
NBT = BC // 128           # 16 batch tiles per core
COV = 256
H = 512
DD = 25
KNOTS = [0.33, 0.66]
DEGREE = 2

_BF16 = ml_dtypes.bfloat16
_FP8 = ml_dtypes.float8_e4m3

_CACHE = {}


def _treat_basis(t):
    """Truncated power basis + kron, matching the reference (fp32 numpy)."""
    t = t.astype(np.float32)
    knots = np.asarray([KNOTS, KNOTS], dtype=np.float32)              # [2, K]
    powers = np.stack([t ** p for p in range(DEGREE + 1)], axis=-1)   # [B, 2, 3]
    rel = np.maximum(t[..., None] - knots[None], 0.0) ** DEGREE       # [B, 2, 2]
    basis = np.concatenate([powers, rel], axis=-1)                    # [B, 2, 5]
    tb = np.einsum('bi,bj->bij', basis[:, 0], basis[:, 1])
    return tb.reshape(t.shape[0], -1)                                 # [B, 25]


def _build_nc():
    if "nc" in _CACHE:
        return _CACHE["nc"]
    from concourse import bacc, mybir
    import concourse.tile as tile

    BF16 = mybir.dt.bfloat16
    FP8 = mybir.dt.float8e4
    F32 = mybir.dt.float32
    AF = mybir.ActivationFunctionType
    OP = mybir.AluOpType
    DR = mybir.MatmulPerfMode.DoubleRow
    F16 = mybir.dt.float16

    nc = bacc.Bacc("TRN2", target_bir_lowering=False, debug=False,
                   num_devices=NCORES)

    xt_d = nc.dram_tensor("xt", [2, 128, BC], BF16, kind="ExternalInput").ap()
    w1_d = nc.dram_tensor("w1", [2, 128, H], BF16, kind="ExternalInput").ap()
    w2_d = nc.dram_tensor("w2", [4, 128, H], BF16, kind="ExternalInput").ap()
    b1_d = nc.dram_tensor("b1c", [128, 4], F32, kind="ExternalInput").ap()
    # b2c columns 0-3: b2; columns 4-7: 256*b2 (bias for the fp8-scaled copy)
    b2_d = nc.dram_tensor("b2c", [128, 8], F32, kind="ExternalInput").ap()
    # dfc1_w in DoubleRow-interleaved fp8: [icp, p, j, d, o], k = icp*256+j*128+p
    wt_d = nc.dram_tensor("wt", [2, 128, 2, DD, H], FP8, kind="ExternalInput").ap()
    d1b_d = nc.dram_tensor("d1bT", [DD, H], BF16, kind="ExternalInput").ap()
    tbp_d = nc.dram_tensor("tbp", [128, NBT, DD + 1], F32, kind="ExternalInput").ap()
    tbt_d = nc.dram_tensor("tbt", [DD, BC], BF16, kind="ExternalInput").ap()
    w2d_d = nc.dram_tensor("w2d", [DD, 516], BF16, kind="ExternalInput").ap()

    hid_d = nc.dram_tensor("hid", [4, 128, BC], BF16, kind="ExternalOutput").ap()
    q_d = nc.dram_tensor("q", [128, NBT], F32, kind="ExternalOutput").ap()

    # d-group schedule for the dfc1 accumulation: 26 psum slots
    # (1 bias + 25 basis indices) in groups of 2 (one [128, 1024] psum tile
    # = 2 banks, triple-buffered = 6 banks; 2 banks left for the u matmuls).
    # The drain chain is 3 stages deep (PE fill -> ACT scaled copy -> DVE add),
    # so triple buffering is needed to hide the chain latency.
    # tb[:, 0] == 1 identically, so the dfc1 bias matmul (x256 on host) is
    # accumulated straight into slot d=0 and picked up by its 1/256 drain.
    slots = list(range(DD))              # 25 slots, bias folded into slot 0
    groups = [slots[i:i + 2] for i in range(0, DD, 2)]   # 12*2 + 1
    # which groups keep a fused DVE scalar_tensor_tensor drain (6 of 13);
    # the rest drain via ScalarE scaled-copy + DVE fp16 add (19 ACT slots)
    dve_group = [1, 0, 1, 0, 1, 0, 1, 0, 1, 0, 1, 0, 1]

    with tile.TileContext(nc) as tc:
        with (
            tc.tile_pool(name="const", bufs=1) as cp,
            tc.tile_pool(name="work", bufs=2) as wp,
        ):
            # small tensors first so phase 1 / phase 2 heads are not stuck
            # behind the 13 MB dfc1 weight load
            w1 = cp.tile([128, 2, H], BF16)
            for ic in range(2):
                nc.sync.dma_start(w1[:, ic, :], w1_d[ic])
            xt = cp.tile([128, 2, BC], BF16)
            for bc_i in range(4):
                for ic in range(2):
                    nc.sync.dma_start(xt[:, ic, bc_i * 512:(bc_i + 1) * 512],
                                      xt_d[ic][:, bc_i * 512:(bc_i + 1) * 512])
            w2 = cp.tile([128, 4, H], BF16)
            for ic in range(4):
                nc.sync.dma_start(w2[:, ic, :], w2_d[ic])
            b1s = cp.tile([128, 4], F32)
            nc.sync.dma_start(b1s[:], b1_d)
            b2s = cp.tile([128, 8], F32)
            nc.sync.dma_start(b2s[:], b2_d)
            d1b = cp.tile([DD, H], BF16)
            nc.sync.dma_start(d1b[:], d1b_d)
            tbp = cp.tile([128, NBT, DD + 1], F32)
            nc.sync.dma_start(tbp[:], tbp_d)
            tbt = cp.tile([DD, BC], BF16)
            nc.sync.dma_start(tbt[:], tbt_d)
            w2d = cp.tile([DD, 516], BF16)
            nc.sync.dma_start(w2d[:], w2d_d)
            # dfc1 weights, split by d-block (block-major order) so the first
            # d-groups of phase 2 can start before the whole 6.5 MB arrives
            DBLK = [(0, 7), (7, 13), (13, 19), (19, 25)]
            wt_t = {}
            for bi, (d0, d1_) in enumerate(DBLK):
                for icp in range(2):
                    w_ic = cp.tile([128, 2, d1_ - d0, H], FP8, tag=f"wt{icp}_{bi}")
                    nc.sync.dma_start(w_ic[:], wt_d[icp][:, :, d0:d1_, :])
                    wt_t[(icp, bi)] = w_ic

            def wt_rhs(icp, d):
                for bi, (d0, d1_) in enumerate(DBLK):
                    if d0 <= d < d1_:
                        return wt_t[(icp, bi)][:, :, d - d0, :]
                raise AssertionError

            ht = cp.tile([128, 4, BC], BF16)
            ht8 = cp.tile([128, 4, BC], FP8)   # 256*hidden, DoubleRow lhsT
            qout = cp.tile([128, NBT], F32)

            # single psum pool shared by both phases (no pool boundary, so the
            # scheduler can overlap the phase-1 tail with the phase-2 head)
            with (
                tc.tile_pool(name="psum", bufs=3, space="PSUM") as pp,
                tc.tile_pool(name="psumu", bufs=2, space="PSUM") as ppu,
            ):
                # ---- phase 1: representation MLP (transposed activations) ----
                # h1t lives in its own pool that closes after layer 2, so the
                # phase-2 accumulators reuse its SBUF space.
                with tc.tile_pool(name="ph1", bufs=1) as ph1:
                    h1t = ph1.tile([128, 4, BC], BF16)
                    with nc.named_scope("mlp1"):
                        for bc_i in range(4):
                            bsl = slice(bc_i * 512, (bc_i + 1) * 512)
                            for ot in range(4):
                                osl = slice(ot * 128, (ot + 1) * 128)
                                ps = pp.tile([128, 1024], F32, tag="ps")
                                for ic in range(2):
                                    nc.tensor.matmul(ps[:, 0:512], w1[:, ic, osl],
                                                     xt[:, ic, bsl],
                                                     start=(ic == 0), stop=(ic == 1))
                                if ot % 2 == 0:
                                    nc.scalar.activation(
                                        h1t[:, ot, bsl], ps[:, 0:512],
                                        AF.Relu, bias=b1s[:, ot:ot + 1])
                                else:
                                    # relu(psum + b1) on DVE: (ps add b1) max 0
                                    nc.vector.tensor_scalar(
                                        h1t[:, ot, bsl], ps[:, 0:512],
                                        b1s[:, ot:ot + 1], 0.0,
                                        OP.add, OP.max)
                    with nc.named_scope("mlp2"):
                        for bc_i in range(4):
                            bsl = slice(bc_i * 512, (bc_i + 1) * 512)
                            for ot in range(4):
                                osl = slice(ot * 128, (ot + 1) * 128)
                                ps = pp.tile([128, 1024], F32, tag="ps")
                                for ic in range(4):
                                    nc.tensor.matmul(ps[:, 0:512], w2[:, ic, osl],
                                                     h1t[:, ic, bsl],
                                                     start=(ic == 0), stop=(ic == 3))
                                nc.vector.tensor_scalar(
                                    ht[:, ot, bsl], ps[:, 0:512],
                                    b2s[:, ot:ot + 1], 0.0, OP.add, OP.max)
                                nc.scalar.activation(ht8[:, ot, bsl], ps[:, 0:512],
                                                     AF.Relu, scale=256.0,
                                                     bias=b2s[:, 4 + ot:5 + ot])
                                nc.sync.dma_start(hid_d[ot][:, bsl],
                                                  ht[:, ot, bsl])

                # ---- phase 2: dynamic FC layers, d-group-major ----
                # All 16 batch tiles advance one d-group at a time, so group g
                # only needs d-block g's weights: compute overlaps the 13 MB
                # dfc1 weight stream instead of stalling on it.
                with tc.tile_pool(name="paccs", bufs=1) as paccs:
                    accs = paccs.tile([128, NBT, 512], F16)
                    with nc.named_scope("dfc"):
                        for gi, group in enumerate(groups):
                            # drain split: one d per (full) group stays a fused
                            # DVE scalar_tensor_tensor; the others go to
                            # ScalarE (scaled psum->fp16 copy) + cheap fp16
                            # 2x-mode DVE adds. 8 DVE-fused + 18 ACT slots
                            # balances ACT/DVE against the PE fill rate.
                            dve_d = group[-1] if dve_group[gi] else None
                            for bt in range(NBT):
                                bsl = slice(bt * 128, (bt + 1) * 128)
                                acc = accs[:, bt, :]
                                ps = pp.tile([128, 1024], F32, tag="ps")
                                for icp in range(2):
                                    for j, d in enumerate(group):
                                        psl = ps[:, j * 512:(j + 1) * 512]
                                        if d == 0 and icp == 0:
                                            # 256*dfc1_b, same 1/256 drain scale
                                            nc.tensor.matmul(
                                                psl, tbt[:, bsl], d1b[:],
                                                start=True, stop=False)
                                        nc.tensor.matmul(
                                            psl,
                                            ht8[:, 2 * icp:2 * icp + 2, bsl],
                                            wt_rhs(icp, d),
                                            start=(icp == 0 and d != 0),
                                            stop=(icp == 1),
                                            perf_mode=DR)
                                tmps = []
                                for j, d in enumerate(group):
                                    psl = ps[:, j * 512:(j + 1) * 512]
                                    if d != dve_d:
                                        tmp = wp.tile([128, 512], F16,
                                                      tag=f"tmp{j}")
                                        nc.scalar.activation(
                                            tmp[:], psl, AF.Copy,
                                            scale=tbp[:, bt, d:d + 1])
                                        tmps.append((j, tmp))
                                if gi == 0 and bt == 0:
                                    pass
                                for j, d in enumerate(group):
                                    if d == dve_d:
                                        psl = ps[:, j * 512:(j + 1) * 512]
                                        if gi == 0:
                                            # first touch of acc this bt
                                            nc.vector.tensor_scalar(
                                                acc, psl, tbp[:, bt, d:d + 1],
                                                None, OP.mult)
                                        else:
                                            nc.vector.scalar_tensor_tensor(
                                                acc, psl, tbp[:, bt, d:d + 1],
                                                acc, OP.mult, OP.add)
                                first = (gi == 0 and not dve_group[0])
                                for j, tmp in tmps:
                                    if first:
                                        nc.vector.tensor_copy(acc, tmp[:])
                                        first = False
                                    else:
                                        nc.vector.tensor_add(acc, acc, tmp[:])
                                if gi == len(groups) - 1:
                                    # Q epilogue, fused so it overlaps the
                                    # remaining batch tiles' last groups
                                    h1b = wp.tile([128, 512], F16, tag="h1b")
                                    nc.scalar.activation(h1b[:], acc, AF.Relu)
                                    ups = ppu.tile([128, 512], F32, tag="ups")
                                    nc.tensor.matmul(ups[:], tbt[:, bsl],
                                                     w2d[:, 0:512],
                                                     start=True, stop=True)
                                    vt = wp.tile([128, 512], F32, tag="vt")
                                    qraw = wp.tile([128, 1], F32, tag="qraw")
                                    nc.vector.scalar_tensor_tensor(
                                        vt[:], ups[:], 1.0, h1b[:],
                                        OP.mult, OP.mult, accum_out=qraw[:])
                                    nc.vector.tensor_add(qout[:, bt:bt + 1],
                                                         qraw[:],
                                                         tbp[:, bt, DD:DD + 1])
                        nc.sync.dma_start(q_d[:], qout[:])

    nc.finalize()
    _CACHE["nc"] = nc
    return nc


def _prepare(t, x, W1, b1, W2, b2, dfc1_w, dfc1_b, dfc2_w, dfc2_b):
    t = np.asarray(t, dtype=np.float32)
    x = np.asarray(x, dtype=np.float32)
    W1 = np.asarray(W1, dtype=np.float32)
    b1 = np.asarray(b1, dtype=np.float32)
    W2 = np.asarray(W2, dtype=np.float32)
    b2 = np.asarray(b2, dtype=np.float32)
    dfc1_w = np.asarray(dfc1_w, dtype=np.float32)
    dfc1_b = np.asarray(dfc1_b, dtype=np.float32)
    dfc2_w = np.asarray(dfc2_w, dtype=np.float32)
    dfc2_b = np.asarray(dfc2_b, dtype=np.float32)

    # replicated weights (host-side relayouts)
    w1 = np.ascontiguousarray(W1.reshape(2, 128, H)).astype(_BF16)
    w2 = np.ascontiguousarray(W2.reshape(4, 128, H)).astype(_BF16)
    b1c = np.ascontiguousarray(b1.reshape(4, 128).T)
    b2q = b2.reshape(4, 128).T
    b2c = np.ascontiguousarray(np.concatenate([b2q, 256.0 * b2q], axis=1))
    # DoubleRow-interleaved fp8 dfc1_w: [icp, p, j, d, o], k = icp*256+j*128+p
    wt = np.ascontiguousarray(
        dfc1_w.reshape(2, 2, 128, H, DD).transpose(0, 2, 1, 4, 3)
    ).astype(_FP8)
    d1bT = np.ascontiguousarray(256.0 * dfc1_b.T).astype(_BF16)  # [25, 512]
    w2d = np.zeros((DD, 516), np.float32)
    w2d[:, :512] = dfc2_w[:, 0, :].T
    w2d[:, 512] = dfc2_b[0]
    w2d = w2d.astype(_BF16)

    tb = _treat_basis(t)                                          # [B, 25] f32

    in_maps = []
    for c in range(NCORES):
        rs = slice(c * BC, (c + 1) * BC)
        xs = x[rs]
        tbs = tb[rs]
        in_maps.append(dict(
            xt=np.ascontiguousarray(xs.T.reshape(2, 128, BC)).astype(_BF16),
            w1=w1, w2=w2, b1c=b1c, b2c=b2c, wt=wt, d1bT=d1bT,
            tbp=np.concatenate([
                np.ascontiguousarray(
                    tbs.reshape(NBT, 128, DD).transpose(1, 0, 2)) / 256.0,
                (tbs @ dfc2_b[0]).reshape(NBT, 128, 1).transpose(1, 0, 2),
            ], axis=2),
            tbt=np.ascontiguousarray(tbs.T).astype(_BF16),
            w2d=w2d,
        ))
    return in_maps


def _gather(res):
    Q = np.empty((B, 1), np.float32)
    hidden = np.empty((B, H), np.float32)
    for c in range(NCORES):
        rs = slice(c * BC, (c + 1) * BC)
        hid = res.results[c]["hid"]                # [4, 128, BC] bf16
        hidden[rs] = hid.reshape(H, BC).T.astype(np.float32)
        q = res.results[c]["q"]                    # [128, NBT]
        Q[rs, 0] = q.T.reshape(BC)
    return (Q, hidden)


def kernel(t, x, W1, b1, W2, b2, dfc1_w, dfc1_b, dfc2_w, dfc2_b):
    from concourse.bass_utils import run_bass_kernel_spmd

    in_maps = _prepare(t, x, W1, b1, W2, b2, dfc1_w, dfc1_b, dfc2_w, dfc2_b)
    nc = _build_nc()
    res = run_bass_kernel_spmd(nc, in_maps, core_ids=list(range(NCORES)))
    return _gather(res)
